# revision 37
# baseline (speedup 1.0000x reference)
"""BERT multi-head self-attention on 8 Trainium2 NeuronCores.

Problem: B=2, S=2048, H=768, NH=12, HD=64 (fp32 reference).

Sharding (hardcoded): core c in 0..7 handles batch b=c//4 and head group
g=c%4 (heads 3g..3g+2).  Each core computes its 3 heads' attention plus the
partial output projection; the host sums the 4 partial outputs per batch
element and adds the (bv @ Wo + bo) constant row (bv passes through softmax
unchanged; bk shifts every score of a softmax row equally and cancels).

Device pipeline per core (cost-model-driven):
  1. QKV projections as fp8e4 DoubleRow matmuls (x and 64x-scaled weights
     split hi/lo into e4m3 planes; 3 cross terms, 256-deep contractions).
  2. Q evicted as fp8 hi/lo planes, K as single fp8: scoresT[k, q] computed
     with ONE DoubleRow matmul per 512 columns - stationary K broadcast over
     both planes, moving Q = (hi, lo) - at 0.5 PE cycles/row, i.e. half the
     bf16 cost, with only K's fp8 quantization as extra error (~7e-3 final).
  3. exp split across engines: most tiles on ACT (exp activation straight
     from PSUM); the rest copied PSUM->SBUF by DVE and raised via
     (e^0.125)^s with a GPSIMD pow (bitwise-exact fp32), balancing the three
     non-PE engines.
  4. PV in the [q, d] layout: ctx[q, d] += P_kb^T @ V_kb pays only 64
     output rows per accumulation step (vs 2048 in the [d, q] layout);
     heads 0/1 share one PSUM tile (columns 0:64 / 64:128).  The softmax
     denominator accumulates via 1-column matmuls against a ones vector.
  5. ctx normalized on GPSIMD (per-partition 1/D scalars from a DVE
     reciprocal), laid out in 128-column transpose blocks [h0-d|h1-d] and
     [h2-d|pad], then PE-transposed (one dummy start=True transpose zeroes
     the shared PSUM bank, the real transposes accumulate with start=False)
     into op-ready stacked lhsT tiles.  A start=True matmul resets its
     whole 2KB PSUM bank on TRN2, so every sub-bank accumulator here (PV
     and transposes) is opened by one bank-wide dummy and accumulated with
     start=False.
  6. out[q, :] = ctxT01^T @ Wo01 + ctxT2^T @ Wo2 per 128-row q-block with a
     128-deep first contraction (two heads per instruction).

Schedule: one flat software pipeline over all 96 (phase, k-block) score
tiles (phase = (q-half, head)), with a LAG-deep pending queue so phase
boundaries cost nothing; projection and V units drain as deadline-ordered
fillers inside the first phase; the qh0 transpose and output projection are
hooked into the stream strictly after finish_phase(2) drains (at global
push index 47 + LAG - issue order IS the synchronization contract, reads
issued before their writer exist race on first execution); only the second
q-half's op stage is a tail.
"""

import os
import sys
import numpy as np

for _p in ("/opt/trn_rl_repo",):
    if _p not in sys.path and os.path.isdir(_p):
        sys.path.append(_p)

import ml_dtypes  # noqa: E402

from concourse import bacc  # noqa: E402
import concourse.mybir as mybir  # noqa: E402
import concourse.tile as tile  # noqa: E402
from concourse.bass_utils import run_bass_kernel_spmd  # noqa: E402

B, S, H = 2, 2048, 768
NH, HD = 12, 64
HPC = 3
NCORES = 8
P = 128
NKB = S // P
NQB = S // P
NHC = H // P
NCP = NHC // 2
QH = 1024
NQH = S // QH
QBH = QH // P            # q-blocks per half (8)
F32 = mybir.dt.float32
F16 = mybir.dt.float16
F8 = mybir.dt.float8e4

NP_F16 = np.float16
NP_F8 = ml_dtypes.float8_e4m3

WS = 64.0
EBASE = float(np.exp(0.125))
DEBUG_TAPS = False
LAG = 10
PT_BUFS = 12
DR = mybir.MatmulPerfMode.DoubleRow

# exp tiles routed DVE-copy + GPSIMD-pow instead of ACT, per phase 0..5
# (phase = (qh, h) in row-major order).  Avoid kb >= 13: the pool path has
# ~2.7us latency and would stall the phase boundary.
POOL_KBS = [
    {5, 9},
    {0, 3, 6, 9, 12},
    {0, 3, 6, 9, 12},
    {0, 3, 6, 9, 12},
    {0, 2, 4, 6, 8, 10, 12},
    {0, 2, 4, 6, 8, 10, 12},
]


def _build_nc(use_mask: bool, use_bias: bool):
    import contextlib
    from collections import deque

    nc = bacc.Bacc("TRN2", target_bir_lowering=False)
    AF = mybir.ActivationFunctionType
    ALU = mybir.AluOpType
    MULT, ADD, SUB = ALU.mult, ALU.add, ALU.subtract

    xt8h = nc.dram_tensor("xt8h", [H, S], F8, kind="ExternalInput")
    xt8l = nc.dram_tensor("xt8l", [H, S], F8, kind="ExternalInput")
    wqk8h = nc.dram_tensor("wqk8h", [P, NHC, 2 * HPC * HD], F8,
                           kind="ExternalInput")
    wqk8l = nc.dram_tensor("wqk8l", [P, NHC, 2 * HPC * HD], F8,
                           kind="ExternalInput")
    wv8h = nc.dram_tensor("wv8h", [P, NHC, HPC * HD], F8,
                          kind="ExternalInput")
    wv8l = nc.dram_tensor("wv8l", [P, NHC, HPC * HD], F8,
                          kind="ExternalInput")
    wo01d = nc.dram_tensor("wo01", [P, H], F16, kind="ExternalInput")
    identd = nc.dram_tensor("ident", [P, P], F16, kind="ExternalInput")
    wo2d = nc.dram_tensor("wo2", [HD, H], F16, kind="ExternalInput")
    if use_bias:
        bqk = nc.dram_tensor("bqk", [2 * HPC * HD, 1], F32, kind="ExternalInput")
    if use_mask:
        mv = nc.dram_tensor("mv", [S, 1], F32, kind="ExternalInput")
    out = nc.dram_tensor("out", [S, H], F16, kind="ExternalOutput")
    if DEBUG_TAPS:
        dbg_qhl = nc.dram_tensor("dbg_qhl", [P, 2, S], F8, kind="ExternalOutput")
        dbg_k01 = nc.dram_tensor("dbg_k01", [P, S], F8, kind="ExternalOutput")
        dbg_m2 = nc.dram_tensor("dbg_m2", [P, 2, S], F8, kind="ExternalOutput")
        dbg_k2s = nc.dram_tensor("dbg_k2s", [HD, S], F8, kind="ExternalOutput")
        dbg_pt = nc.dram_tensor("dbg_pt", [P, QH], F16, kind="ExternalOutput")
        dbg_v = nc.dram_tensor("dbg_v", [P, NKB, HPC, HD + 1], F16,
                               kind="ExternalOutput")
        dbg_cn = nc.dram_tensor("dbg_cn", [P, QBH, 2, P], F16,
                                kind="ExternalOutput")

    with tile.TileContext(nc) as tc, contextlib.ExitStack() as ctx, \
            nc.allow_low_precision(reason="fp8/fp16 compute pipeline by design"):
        const = ctx.enter_context(tc.tile_pool(name="const", bufs=1))
        xt_pool = ctx.enter_context(tc.tile_pool(name="xt", bufs=1))
        w_pool = ctx.enter_context(tc.tile_pool(name="w", bufs=1))
        qk_pool = ctx.enter_context(tc.tile_pool(name="qk", bufs=1))
        v_pool = ctx.enter_context(tc.tile_pool(name="v", bufs=1))
        pt_pool = ctx.enter_context(tc.tile_pool(name="pt", bufs=PT_BUFS))
        ssb_pool = ctx.enter_context(tc.tile_pool(name="ssb", bufs=4))
        cu_pool = ctx.enter_context(tc.tile_pool(name="cu", bufs=3))
        cn_pool = ctx.enter_context(tc.tile_pool(name="cn", bufs=1))
        osb_pool = ctx.enter_context(tc.tile_pool(name="osb", bufs=4))

        # ---- constants ----
        ebase_t = const.tile([P, 1], F32, tag="ebase")
        nc.vector.memset(ebase_t[:], EBASE)
        ones16_t = const.tile([P, 1], F16, tag="ones16")
        nc.vector.memset(ones16_t[:], 1.0)
        zeros16_t = const.tile([P, 512], F16, tag="zeros16")
        nc.vector.memset(zeros16_t[:], 0.0)
        ident_t = const.tile([P, P], F16, tag="ident")

        # ---- weight / x loads ----
        wqk8h_t = w_pool.tile([P, NHC, 2 * HPC * HD], F8, tag="wqk8h")
        wqk8l_t = w_pool.tile([P, NHC, 2 * HPC * HD], F8, tag="wqk8l")
        wv8h_t = w_pool.tile([P, NHC, HPC * HD], F8, tag="wv8h")
        wv8l_t = w_pool.tile([P, NHC, HPC * HD], F8, tag="wv8l")
        nc.sync.dma_start(wqk8h_t[:], wqk8h[:])
        nc.scalar.dma_start(wqk8l_t[:], wqk8l[:])
        xt8h_t = xt_pool.tile([P, NHC, S], F8, tag="xt8h")
        xt8l_t = xt_pool.tile([P, NHC, S], F8, tag="xt8l")

        def xt_load(qs):
            for cp in range(NCP):
                rows = slice(2 * cp * P, (2 * cp + 2) * P)
                cs = slice(2 * cp, 2 * cp + 2)
                nc.sync.dma_start(
                    xt8h_t[:, cs, qs],
                    xt8h[rows, qs].rearrange("(c p) n -> p c n", p=P))
                nc.scalar.dma_start(
                    xt8l_t[:, cs, qs],
                    xt8l[rows, qs].rearrange("(c p) n -> p c n", p=P))

        xt_load(slice(0, QH))
        xt8h_p = [xt8h_t[:, 2 * cp:2 * cp + 2, :] for cp in range(NCP)]
        xt8l_p = [xt8l_t[:, 2 * cp:2 * cp + 2, :] for cp in range(NCP)]
        xt_load(slice(QH, S))
        nc.scalar.dma_start(wv8h_t[:], wv8h[:])
        nc.scalar.dma_start(wv8l_t[:], wv8l[:])
        nc.scalar.dma_start(ident_t[:], identd[:])
        wo01_t = w_pool.tile([P, H], F16, tag="wo01")
        wo2_t = w_pool.tile([HD, H], F16, tag="wo2")
        nc.scalar.dma_start(wo01_t[:], wo01d[:])
        nc.scalar.dma_start(wo2_t[:], wo2d[:])
        if use_bias:
            bias_t = const.tile([P, 3], F32, tag="bqk")
            nc.sync.dma_start(
                bias_t[:], bqk[:].rearrange("(m p) one -> p (m one)", p=P))
            bias_sb = [bias_t[:, m:m + 1] for m in range(3)]
        if use_mask:
            mvf_t = const.tile([P, NKB], F32, tag="mvf")
            nc.scalar.dma_start(
                mvf_t[:], mv[:].rearrange("(kb p) one -> p (kb one)", p=P))
            mv16_t = const.tile([P, NKB], F16, tag="mv16")
            nc.vector.tensor_copy(mv16_t[:], mvf_t[:])

        # ---- Q/K/V sbuf tiles ----
        qhl01_t = qk_pool.tile([P, 2, S], F8, tag="qhl01")   # Q h0,h1 hi/lo
        m2hl_t = qk_pool.tile([P, 2, S], F8, tag="m2hl")     # Q2 hi/lo | K2,-
        k01_t = qk_pool.tile([P, S], F8, tag="k01")          # K h0,h1
        k2s_t = qk_pool.tile([HD, S], F8, tag="k2s")         # K2 shifted
        if use_bias:
            qtmp_t = qk_pool.tile([P, S], F16, tag="qtmp")
        v_t = v_pool.tile([P, NKB, HPC, HD + 1], F16, tag="v")

        # ---- context tiles ----
        recip_t = const.tile([P, HPC, NQB], F32, tag="recip")
        # ctxn layout per q-half: [q-part, qb, block-type, 128]
        # type 0 = [h0-d | h1-d], type 1 = [h2-d | pad]
        ctxn_t = [cn_pool.tile([P, QBH, 2, P], F16, tag=f"ctxn{qh}",
                               name=f"ctxn{qh}") for qh in range(NQH)]
        ctxT_t = [cn_pool.tile([P, 2 * QBH, P], F16, tag=f"ctxT{qh}",
                               name=f"ctxT{qh}") for qh in range(NQH)]
        for qh in range(NQH):
            # the h2 transpose blocks are half pad; zero them once so the
            # xbar transpose never reads uninitialized SBUF
            nc.gpsimd.memset(ctxn_t[qh][:, :, 1, HD:P], 0.0)

        # order: the two @wh terms first (startable with only wqk8h
        # loaded), the @wl term last so the wqk8l DMA leaves the startup
        # critical path
        HILO = ((xt8h_p, wqk8h_t, wv8h_t), (xt8l_p, wqk8h_t, wv8h_t),
                (xt8h_p, wqk8l_t, wv8l_t))

        with tc.tile_pool(name="sc", bufs=3, space="PSUM") as sc_psum, \
             tc.tile_pool(name="ctxp", bufs=1, space="PSUM") as ctx_psum:

            # ones column of every V_aug tile (feeds the D accumulator);
            # the mask path overwrites it per kb inside v_unit
            if not use_mask:
                nc.vector.memset(v_t[:, :, :, HD:HD + 1], 1.0)

            def borrow_psum(cols, name):
                ps = sc_psum.tile([P, QH], F32, tag="sc", name=name)
                return ps[:, 0:cols]

            # ---------------- projection units ----------------
            def qkt_unit(qc, m):
                qs = slice(qc * 512, (qc + 1) * 512)
                ms = slice(m * P, (m + 1) * P)
                ps = borrow_psum(512, "pjps")
                for t, (xa, wa, _) in enumerate(HILO):
                    for cp in range(NCP):
                        cs = slice(2 * cp, 2 * cp + 2)
                        nc.tensor.matmul(
                            ps[:], wa[:, cs, ms], xa[cp][:, :, qs],
                            start=(t == 0 and cp == 0),
                            stop=(t == 2 and cp == NCP - 1),
                            perf_mode=DR,
                        )
                bias = bias_sb[m][:] if use_bias else None
                if m == 1:       # K heads 0,1: single fp8
                    if bias is None:
                        nc.vector.tensor_scalar_mul(k01_t[:, qs], ps[:],
                                                    1.0 / WS)
                    else:
                        nc.vector.tensor_scalar(k01_t[:, qs], ps[:],
                                                1.0 / WS, bias, MULT, ADD)
                    return
                dst = qhl01_t if m == 0 else m2hl_t
                lo_parts = slice(0, P) if m == 0 else slice(0, HD)
                if bias is None:
                    nc.vector.tensor_scalar_mul(dst[:, 0, qs], ps[:], 1.0 / WS)
                    nc.vector.scalar_tensor_tensor(
                        dst[lo_parts, 1, qs], ps[lo_parts, :], 1.0 / WS,
                        dst[lo_parts, 0, qs], MULT, SUB)
                else:
                    nc.vector.tensor_scalar(dst[:, 0, qs], ps[:], 1.0 / WS,
                                            bias, MULT, ADD)
                    nc.vector.tensor_scalar(qtmp_t[:, qs], ps[:], 1.0 / WS,
                                            bias, MULT, ADD)
                    nc.vector.tensor_tensor(
                        dst[lo_parts, 1, qs], qtmp_t[lo_parts, qs],
                        dst[lo_parts, 0, qs], SUB)
                if m == 2 and qc == 3:
                    # K2 partition shift 64->0, one DMA once all of m2 landed
                    nc.sync.dma_start(k2s_t[:], m2hl_t[HD:P, 0, :])

            def v_unit(kb):
                ks = slice(kb * P, (kb + 1) * P)
                ps = borrow_psum(HPC * HD, "vps")
                for t, (xa, _, va) in enumerate(HILO):
                    for cp in range(NCP):
                        cs = slice(2 * cp, 2 * cp + 2)
                        nc.tensor.matmul(
                            ps[:], xa[cp][:, :, ks], va[:, cs, :],
                            start=(t == 0 and cp == 0),
                            stop=(t == 2 and cp == NCP - 1),
                            perf_mode=DR,
                        )
                if use_mask:
                    nc.vector.tensor_scalar(
                        v_t[:, kb, :, 0:HD],
                        ps[:].rearrange("p (h d) -> p h d", h=HPC),
                        1.0 / WS, mvf_t[:, kb:kb + 1], MULT, MULT)
                    nc.vector.tensor_scalar_mul(
                        v_t[:, kb, :, HD:HD + 1],
                        ones16_t[:, None, :].to_broadcast((P, HPC, 1)),
                        mvf_t[:, kb:kb + 1])
                else:
                    nc.vector.tensor_scalar_mul(
                        v_t[:, kb, :, 0:HD],
                        ps[:].rearrange("p (h d) -> p h d", h=HPC), 1.0 / WS)

            # ---------------- attention helpers ----------------
            def k_ap(h, kb):
                ks = slice(kb * P, (kb + 1) * P)
                if h < 2:
                    base = k01_t[h * HD:(h + 1) * HD, ks]
                else:
                    base = k2s_t[:, ks]
                return base[:, None, :].to_broadcast((HD, 2, P))

            def q_ap(h, qh, c):
                qs = slice(qh * QH + c * 512, qh * QH + (c + 1) * 512)
                if h < 2:
                    return qhl01_t[h * HD:(h + 1) * HD, :, qs]
                return m2hl_t[0:HD, :, qs]

            fillers = deque()

            def drain(n=1):
                for _ in range(n):
                    if fillers:
                        fillers.popleft()()

            def transpose_qh(qh, part=None):
                # PE transposes; the f16 [P, 8, P] batch lives in ONE PSUM
                # bank, and any start=True write resets the whole bank - so
                # one dummy transpose zeroes it, then the real transposes
                # accumulate with start=False.
                parts = (0, 1) if part is None else (part,)
                for pa in parts:
                    tps = sc_psum.tile([P, QBH, P], F16, tag="sc",
                                       name="tps")
                    nc.tensor.matmul(tps[:, 0, :], zeros16_t[0:P, 0:P],
                                     ident_t[:], is_transpose=True,
                                     start=True, stop=False,
                                     skip_group_check=True)
                    for j in range(QBH):
                        blk = pa * QBH + j
                        nc.tensor.matmul(
                            tps[:, j, :],
                            ctxn_t[qh][:, blk // 2, blk % 2, :],
                            ident_t[:], is_transpose=True,
                            start=False, stop=True, skip_group_check=True)
                    nc.vector.tensor_copy(
                        ctxT_t[qh][:, pa * QBH:(pa + 1) * QBH, :], tps[:])

            osb2 = {}

            def op_unit(qh, qb, evict_act=False):
                if qb % 2 == 0:
                    osb2[qh] = osb_pool.tile([P, 2, H], F16, tag="osb",
                                             name="osb")
                osb = osb2[qh]
                for i, nsl in enumerate((slice(0, 512), slice(512, H))):
                    ops = borrow_psum(nsl.stop - nsl.start, "ops")
                    nc.tensor.matmul(ops[:], ctxT_t[qh][:, 2 * qb, :],
                                     wo01_t[:, nsl], start=True, stop=False)
                    nc.tensor.matmul(ops[:],
                                     ctxT_t[qh][0:HD, 2 * qb + 1, :],
                                     wo2_t[:, nsl], start=False, stop=True)
                    if evict_act and i == 1:
                        nc.scalar.copy(osb[:, qb % 2, nsl], ops[:])
                    else:
                        nc.vector.tensor_copy(osb[:, qb % 2, nsl], ops[:])
                if qb % 2 == 1:
                    # always the sync queue: a DMA's sem-wait parks its
                    # engine SEQ, and the scalar queue would stall the ACT
                    # exp stream behind it
                    qsl = slice(qh * QH + (qb - 1) * P, qh * QH + (qb + 1) * P)
                    nc.sync.dma_start(
                        out[qsl, :].rearrange("(two p) n -> p two n", p=P),
                        osb[:])

            # ---------------- flat pipelined attention stream ----------
            # phase = (qh, h) row-major; one global software pipeline so
            # phase boundaries cost nothing.
            cps_of = {}
            pending = deque()

            def finish_phase(ph):
                qh, h = divmod(ph, HPC)
                cps = cps_of[ph]
                nc.vector.reciprocal(
                    recip_t[:, h, qh * QBH:(qh + 1) * QBH], cps[:, :, HD])
                cu = cu_pool.tile([P, QBH, HD], F16, tag="cu", name="cu")
                nc.vector.tensor_copy(cu[:], cps[:, :, 0:HD])
                for qb in range(QBH):
                    if h == 0:
                        dst = ctxn_t[qh][:, qb, 0, 0:HD]
                    elif h == 1:
                        dst = ctxn_t[qh][:, qb, 0, HD:P]
                    else:
                        dst = ctxn_t[qh][:, qb, 1, 0:HD]
                    eng = nc.vector if (ph == 5 and qb < QBH // 2) \
                        else nc.gpsimd
                    eng.tensor_scalar(
                        dst, cu[:, qb, :],
                        recip_t[:, h, qh * QBH + qb:qh * QBH + qb + 1],
                        None, MULT)
                    if ph == 5 and qb == QBH // 2 - 1:
                        transpose_qh(1, part=0)
                if ph == 5:
                    transpose_qh(1, part=1)

            def zero_cps(cps):
                # start=True resets the full 2KB PSUM bank, so sub-bank
                # accumulators must be opened by one bank-wide dummy matmul
                for half in range(2):
                    nc.tensor.matmul(
                        cps[:, 4 * half:4 * half + 4, :].rearrange(
                            "p a b -> p (a b)"),
                        zeros16_t[0:1, 0:P], zeros16_t[0:1, :],
                        start=True, stop=False, skip_group_check=True)

            def do_pv():
                ph, kb, pt = pending.popleft()
                qh, h = divmod(ph, HPC)
                cps = cps_of[ph]
                if kb == 0:
                    zero_cps(cps)
                for qb in range(QBH):
                    nc.tensor.matmul(
                        cps[:, qb, 0:HD + 1],
                        pt[:, qb * P:(qb + 1) * P], v_t[:, kb, h, :],
                        start=False, stop=(kb == NKB - 1),
                        skip_group_check=True)
                if kb == NKB - 1:
                    finish_phase(ph)

            # prefix: Q01 first half + K01 first 4 kb
            qkt_unit(0, 0)
            qkt_unit(1, 0)
            qkt_unit(0, 1)

            sched = ([("v", 0), ("v", 1), ("v", 2), ("qk", 1, 1),
                      ("v", 3), ("v", 4), ("qk", 2, 1),
                      ("v", 5), ("v", 6), ("qk", 3, 1),
                      ("v", 7), ("v", 8), ("v", 9), ("v", 10), ("v", 11),
                      ("v", 12), ("v", 13), ("v", 14), ("v", 15),
                      ("qk", 0, 2), ("qk", 1, 2), ("qk", 2, 2), ("qk", 3, 2),
                      ("qk", 2, 0), ("qk", 3, 0)])
            for u in sched:
                if u[0] == "v":
                    fillers.append(lambda kb=u[1]: v_unit(kb))
                else:
                    fillers.append(lambda qc=u[1], m=u[2]: qkt_unit(qc, m))

            op_q = deque(range(QBH))
            gi = 0
            for ph in range(2 * HPC):
                qh, h = divmod(ph, HPC)
                pool_set = POOL_KBS[ph]
                for kb in range(NKB):
                    if kb == 0:
                        cps_of[ph] = ctx_psum.tile(
                            [P, QBH, P], F32, tag="ctx", name=f"cps{ph}")
                    sps = sc_psum.tile([P, QH], F32, tag="sc", name="sps")
                    for c in range(QH // 512):
                        nc.tensor.matmul(
                            sps[:, c * 512:(c + 1) * 512],
                            k_ap(h, kb), q_ap(h, qh, c),
                            start=True, stop=True, perf_mode=DR)
                    pt = pt_pool.tile([P, QH], F16, tag="pt", name="pt")
                    if kb in pool_set:
                        sb = ssb_pool.tile([P, QH], F32, tag="ssb",
                                           name="ssb")
                        nc.vector.tensor_copy(sb[:], sps[:])
                        nc.gpsimd.tensor_tensor(
                            pt[:], ebase_t[:].to_broadcast((P, QH)),
                            sb[:], ALU.pow)
                    else:
                        nc.scalar.activation(pt[:], sps[:], AF.Exp,
                                             scale=0.125)
                    if DEBUG_TAPS and ph == 0 and kb == 0:
                        nc.sync.dma_start(dbg_pt[:], pt[:])
                    pending.append((ph, kb, pt))
                    gi += 1
                    drain(2 if gi <= 16 else 1)
                    while len(pending) > LAG:
                        do_pv()
                    # finish_phase(2) is issued by do_pv(ph2, kb15), which
                    # drains at global push index 47 + LAG; the qh0 transpose
                    # and op units must be issued strictly after it
                    t = ph * NKB + kb
                    if t == 48 + LAG:
                        transpose_qh(0)
                    if t >= 50 + LAG and (t - 50 - LAG) % 4 == 0 and op_q:
                        op_unit(0, op_q.popleft())
            while pending:
                do_pv()
            while op_q:
                op_unit(0, op_q.popleft())
            for qb in range(QBH - 2):
                # qb 0-3 only need transpose part 0 (issued mid-finish);
                # part 1 lands while they run
                op_unit(1, qb, evict_act=True)
            for qb in (QBH - 2, QBH - 1):
                osb1 = osb_pool.tile([P, H], F16, tag="osb1", name="osb1")
                for i, nsl in enumerate((slice(0, 512), slice(512, H))):
                    ops = borrow_psum(nsl.stop - nsl.start, "ops")
                    nc.tensor.matmul(ops[:], ctxT_t[1][:, 2 * qb, :],
                                     wo01_t[:, nsl], start=True, stop=False)
                    nc.tensor.matmul(ops[:], ctxT_t[1][0:HD, 2 * qb + 1, :],
                                     wo2_t[:, nsl], start=False, stop=True)
                    if i == 1:
                        nc.scalar.copy(osb1[:, nsl], ops[:])
                    else:
                        nc.vector.tensor_copy(osb1[:, nsl], ops[:])
                qsl = slice(QH + qb * P, QH + (qb + 1) * P)
                eng = nc.sync if qb % 2 == 0 else nc.scalar
                eng.dma_start(out[qsl, :], osb1[:])
            if DEBUG_TAPS:
                nc.sync.dma_start(dbg_qhl[:], qhl01_t[:])
                nc.sync.dma_start(dbg_k01[:], k01_t[:])
                nc.sync.dma_start(dbg_m2[:], m2hl_t[:])
                nc.sync.dma_start(dbg_k2s[:], k2s_t[:])
                nc.sync.dma_start(dbg_v[:], v_t[:])
                nc.sync.dma_start(dbg_cn[:], ctxn_t[0][:])

    nc.compile()
    return nc


_NC_CACHE = {}


def _get_nc(use_mask: bool, use_bias: bool = False):
    key = (use_mask, use_bias)
    if key not in _NC_CACHE:
        _NC_CACHE[key] = _build_nc(use_mask, use_bias)
    return _NC_CACHE[key]


def _hilo(a):
    hi = a.astype(NP_F8)
    lo = (a - hi.astype(np.float32)).astype(NP_F8)
    return hi, lo


def _shard_inputs(hidden_states, attention_mask, Wq, bq, Wk, bk, Wv, bv,
                  Wo, bo, use_mask, use_bias):
    in_maps = []
    for c in range(NCORES):
        b, g = divmod(c, NCORES // B)
        cols = slice(g * HPC * HD, (g + 1) * HPC * HD)
        wq_g = Wq[:, cols]
        wk_g = Wk[:, cols]
        qk_cols = [wq_g[:, 0:HD], wq_g[:, HD:2 * HD],
                   wk_g[:, 0:HD], wk_g[:, HD:2 * HD],
                   wq_g[:, 2 * HD:3 * HD], wk_g[:, 2 * HD:3 * HD]]
        wqk = np.concatenate(qk_cols, axis=1)
        xt = np.ascontiguousarray(hidden_states[b].T).astype(np.float32)
        xt8h, xt8l = _hilo(xt)
        wqk8h, wqk8l = _hilo(np.ascontiguousarray(wqk) * WS)
        wv8h, wv8l = _hilo(np.ascontiguousarray(Wv[:, cols]) * WS)

        def wshape(w):
            # [H, n] -> [P, NHC, n] partition-major for large-elem DMA
            return np.ascontiguousarray(
                w.reshape(NHC, P, -1).transpose(1, 0, 2))

        wqk8h, wqk8l = wshape(wqk8h), wshape(wqk8l)
        wv8h, wv8l = wshape(wv8h), wshape(wv8l)
        wo_g = Wo[cols, :].astype(np.float32)
        m = {
            "xt8h": xt8h, "xt8l": xt8l,
            "wqk8h": wqk8h, "wqk8l": wqk8l,
            "wv8h": wv8h, "wv8l": wv8l,
            "wo01": np.ascontiguousarray(wo_g[0:P, :]).astype(NP_F16),
            "wo2": np.ascontiguousarray(wo_g[P:P + HD, :]).astype(NP_F16),
            "ident": np.eye(P, dtype=NP_F16),
        }
        if use_bias:
            bq_g = bq[cols]
            bk_g = bk[cols]
            bqkv = np.concatenate([bq_g[0:HD], bq_g[HD:2 * HD],
                                   bk_g[0:HD], bk_g[HD:2 * HD],
                                   bq_g[2 * HD:3 * HD], bk_g[2 * HD:3 * HD]])
            m["bqk"] = bqkv.astype(np.float32).reshape(-1, 1)
        if use_mask:
            mvec = np.exp(-10000.0 * (1.0 - attention_mask[b].astype(np.float64)))
            m["mv"] = mvec.astype(np.float32).reshape(-1, 1)
        in_maps.append(m)
    return in_maps


def kernel(hidden_states, attention_mask, Wq, bq, Wk, bk, Wv, bv, Wo, bo):
    hidden_states = np.asarray(hidden_states, np.float32)
    attention_mask = np.asarray(attention_mask)
    Wq, bq = np.asarray(Wq, np.float32), np.asarray(bq, np.float32)
    Wk, bk = np.asarray(Wk, np.float32), np.asarray(bk, np.float32)
    Wv, bv = np.asarray(Wv, np.float32), np.asarray(bv, np.float32)
    Wo, bo = np.asarray(Wo, np.float32), np.asarray(bo, np.float32)

    use_mask = not bool(np.all(attention_mask == 1))
    use_bias = bool(np.any(bq != 0))   # bk cancels in softmax; bv host-folded
    nc = _get_nc(use_mask, use_bias)
    in_maps = _shard_inputs(hidden_states, attention_mask,
                            Wq, bq, Wk, bk, Wv, bv, Wo, bo,
                            use_mask, use_bias)
    res = run_bass_kernel_spmd(nc, in_maps, core_ids=list(range(NCORES)))

    const_row = (bv.astype(np.float64) @ Wo.astype(np.float64)
                 + bo.astype(np.float64))
    out = np.zeros((B, S, H), np.float64)
    for c in range(NCORES):
        b = c // (NCORES // B)
        out[b] += res.results[c]["out"].astype(np.float64)
    out += const_row[None, None, :]
    return out.astype(np.float32)


if __name__ == "__main__":
    rng = np.random.default_rng(0)
    inputs = {
        "hidden_states": rng.standard_normal((B, S, H), np.float32),
        "attention_mask": np.ones((B, S), np.int32),
        "Wq": rng.standard_normal((H, H), np.float32) * 0.02,
        "bq": np.zeros(H, np.float32),
        "Wk": rng.standard_normal((H, H), np.float32) * 0.02,
        "bk": np.zeros(H, np.float32),
        "Wv": rng.standard_normal((H, H), np.float32) * 0.02,
        "bv": np.zeros(H, np.float32),
        "Wo": rng.standard_normal((H, H), np.float32) * 0.02,
        "bo": np.zeros(H, np.float32),
    }
    out = kernel(**inputs)
    print("out", out.shape, out.dtype)
